# revision 1
# baseline (speedup 1.0000x reference)
"""Trainium2 Bass kernel for batched GCN (2x GCNConv + circular Conv1d).

Math per graph (N=64 nodes, S=96 feats, H=512 hidden, E=512 edges):
    deg[d]   = #edges with dst=d (incl. self loop)
    A        = Dinv @ (M0 + I).T @ Dinv,  Dinv = diag(1/sqrt(deg))
    h1       = relu(A @ (x.T @ W1.T) + b1)
    h2       = A @ (h1 @ W2.T) + b2
    y        = circular_conv1d(h2, conv_w).T          # [96, 512]

Device strategy (per core: 64 graphs, processed as 32 pairs of 2 graphs
occupying partition halves 0-63 / 64-127):
  - M0.T built per graph with one-hot matmuls: onehots from a single DVE
    is_equal against an iota table (broadcast APs), then 4 K=128 matmuls
    + identity matmul accumulate counts in PSUM.
  - Row-scale M0.T by dinv (DVE), block-diagonal pair tile, one PE
    transpose -> block-diag Ms = (M_aug @ Dinv) for the pair.
  - agg1 computed transposed (H on partitions) to feed gcn2 without extra
    transposes; dinv folded into z1/z2 copies; bf16 matmuls (fp32 PSUM).
  - conv done as 3 shifted-tap matmuls per graph on a duplicated [h2|h2]
    tile; output lands [l, o]-major in PSUM, copied once, DMA'd out.
"""

import numpy as np
import ml_dtypes

import concourse.bacc as bacc
import concourse.mybir as mybir
import concourse.tile as tile
from concourse.bass_utils import run_bass_kernel_spmd

BF16 = mybir.dt.bfloat16
FP32 = mybir.dt.float32
I32 = mybir.dt.int32
AF = mybir.ActivationFunctionType

N_CORES = 8
B, S, N, H, E = 512, 96, 64, 512, 512
G = B // N_CORES          # graphs per core
NPAIR = G // 2


def build_gcn_kernel(tc, outs, ins, g_per_core=G, has_b1=False, has_b2=False):
    """Emit the kernel into TileContext tc. outs/ins are dicts of DRAM APs."""
    nc = tc.nc
    g = g_per_core
    npair = g // 2

    x_d = ins["x"]          # [g, 96, 64] f32
    ei_d = ins["ei"]        # [2g, 512] i32   (row = 2*graph + (0:src,1:dst))
    w1t_d = ins["w1t"]      # [96, 512] bf16
    w2t_d = ins["w2t"]      # [128, 384] bf16 (f=(c,s))
    cwd_d = ins["cwd"]      # [128, 1536] bf16 (rows 0-63 = [i,(k,o)], dup)
    iota_d = ins["iota"]    # [128, 1024] bf16 (f%64)
    i64d_d = ins["i64d"]    # [128, 64] bf16 (I64 stacked twice)
    id128_d = ins["id128"]  # [128, 128] bf16
    y_d = outs["y"]         # [g, 96, 512] f32

    from contextlib import ExitStack
    ctx = ExitStack()
    const = ctx.enter_context(tc.tile_pool(name="const", bufs=1))
    sb = ctx.enter_context(tc.tile_pool(name="sb", bufs=6))
    psa = ctx.enter_context(tc.tile_pool(name="psa", bufs=3, space="PSUM"))
    ps = ctx.enter_context(tc.tile_pool(name="ps", bufs=2, space="PSUM"))
    ps1 = ctx.enter_context(tc.tile_pool(name="ps1", bufs=1, space="PSUM"))
    psy = ctx.enter_context(tc.tile_pool(name="psy", bufs=2, space="PSUM"))

    # ---- constants ----
    w1t = const.tile([96, 512], BF16)
    nc.sync.dma_start(out=w1t[:], in_=w1t_d[:])
    w2t = const.tile([128, 384], BF16)
    nc.sync.dma_start(out=w2t[:], in_=w2t_d[:])
    cwd = const.tile([128, 1536], BF16)
    nc.sync.dma_start(out=cwd[:], in_=cwd_d[:])
    iota = const.tile([128, 1024], BF16)
    nc.sync.dma_start(out=iota[:], in_=iota_d[:])
    i64d = const.tile([128, 64], BF16)
    nc.sync.dma_start(out=i64d[:], in_=i64d_d[:])
    id128 = const.tile([128, 128], BF16)
    nc.sync.dma_start(out=id128[:], in_=id128_d[:])
    id128f = const.tile([128, 128], FP32)
    nc.vector.tensor_copy(out=id128f[:], in_=id128[:])
    if has_b1:
        b1c = const.tile([128, 4], FP32)
        nc.sync.dma_start(out=b1c[:], in_=ins["b1c"][:])
    if has_b2:
        b2d = const.tile([128, 192], BF16)
        nc.sync.dma_start(out=b2d[:], in_=ins["b2d"][:])

    # ---- x: load + cast to bf16, laid out [s, (g, n)] ----
    xf = const.tile([96, 64 * g], FP32)
    nc.sync.dma_start(out=xf[:].rearrange("s (g n) -> s g n", g=g),
                      in_=x_d[:].rearrange("g s n -> s g n"))
    xbf = const.tile([96, 64 * g], BF16)
    nc.vector.tensor_copy(out=xbf[:], in_=xf[:])

    # ---- edges: load, cast, transpose to [epos, (c, gt)] ----
    ei = const.tile([2 * g, 512], I32)
    nc.sync.dma_start(out=ei[:], in_=ei_d[:])
    eibf = const.tile([2 * g, 512], BF16)
    nc.vector.tensor_copy(out=eibf[:], in_=ei[:])
    et = const.tile([128, 4 * 2 * g], BF16)   # f = (c, gt)
    for c in range(4):
        etp = ps.tile([128, 128], BF16, tag="z1")
        # in_ is [2g, 128] -> out = in_.T = [128, 2g]
        nc.tensor.transpose(
            out=etp[:, 0:2 * g], in_=eibf[:, c * 128:(c + 1) * 128],
            identity=id128[0:2 * g, 0:2 * g],
        )
        nc.scalar.activation(
            out=et[:, c * 2 * g:(c + 1) * 2 * g], in_=etp[:, 0:2 * g],
            func=AF.Copy,
        )

    for pr in range(npair):
        # ---- z1 = x^T W1^T for the pair (independent of A-chain) ----
        z1_ps = ps.tile([128, 512], FP32, tag="z1")
        nc.tensor.matmul(z1_ps[:], xbf[:, 128 * pr:128 * (pr + 1)], w1t[:],
                         start=True, stop=True)

        # ---- one-hots: oh[p, (c, j, v)] = (et[p, (c, 4pr+j)] == v) ----
        e_sl = et[:].rearrange("p (c gt) -> p c gt", c=4)
        e_sl = e_sl[:, :, 4 * pr:4 * pr + 4]
        e_sl = e_sl.rearrange("p c (j u) -> p c j u", u=1)
        e_bc = e_sl.to_broadcast([128, 4, 4, 64])
        erep = sb.tile([128, 1024], BF16, tag="erep")
        nc.gpsimd.tensor_copy(
            out=erep[:].rearrange("p (c j v) -> p c j v", c=4, j=4),
            in_=e_bc)
        oh = sb.tile([128, 1024], BF16, tag="oh")
        nc.vector.tensor_tensor(
            out=oh[:], in0=erep[:], in1=iota[:],
            op=mybir.AluOpType.is_equal,
        )

        # ---- M_aug^T (counts + I) per graph into pair psum [128, 64] ----
        mps = psa.tile([128, 384], FP32, tag="mzz")
        maug = mps[:, 0:64]
        for gl in range(2):
            po = 64 * gl
            out_sl = maug[po:po + 64, :]  # noqa
            tp = None if gl == 0 else (0, 64)
            for c in range(4):
                base = c * 256
                lhsT = oh[:, base + (2 * gl + 1) * 64: base + (2 * gl + 2) * 64]
                rhs = oh[:, base + (2 * gl) * 64: base + (2 * gl + 1) * 64]
                nc.tensor.matmul(out_sl, lhsT, rhs, start=(c == 0),
                                 stop=False, tile_position=tp)
            nc.tensor.matmul(
                out_sl, i64d[po:po + 64, :], i64d[po:po + 64, :],
                start=False, stop=True,
                tile_position=None if gl == 0 else (64, 64),
            )

        # ---- deg -> dinv ----
        deg = sb.tile([128, 1], FP32, tag="deg")
        nc.vector.tensor_reduce(out=deg[:], in_=maug[:, :],
                                axis=mybir.AxisListType.X,
                                op=mybir.AluOpType.add)
        sq = sb.tile([128, 1], FP32, tag="sq")
        nc.scalar.activation(out=sq[:], in_=deg[:], func=AF.Sqrt)
        dinv = sb.tile([128, 1], FP32, tag="dinv")
        nc.vector.reciprocal(out=dinv[:], in_=sq[:])

        # ---- MsT block-diag -> transpose -> Ms block-diag (bf16) ----
        msb = sb.tile([128, 128], FP32, tag="msb")
        nc.gpsimd.memset(msb[:], 0)
        nc.vector.tensor_scalar(
            out=msb[0:64, 0:64], in0=maug[0:64, :], scalar1=dinv[0:64, :],
            scalar2=None, op0=mybir.AluOpType.mult)
        nc.scalar.activation(
            out=msb[64:128, 64:128], in_=maug[64:128, :], func=AF.Copy,
            scale=dinv[64:128, :])
        mst_ps = mps[:, 64:192]
        nc.tensor.transpose(out=mst_ps, in_=msb[:], identity=id128f[:])
        msbd = sb.tile([128, 128], BF16, tag="msbd")
        nc.scalar.activation(out=msbd[:], in_=mst_ps, func=AF.Copy)

        # ---- z1s = dinv * z1 ----
        z1s = sb.tile([128, 512], BF16, tag="z1s")
        nc.vector.tensor_scalar(out=z1s[:], in0=z1_ps[:], scalar1=dinv[:, :],
                                scalar2=None, op0=mybir.AluOpType.mult)

        # ---- agg1T: [128 (h in chunk), (c, g, n)] ----
        a1t_ps = ps1.tile([128, 512], FP32, tag="a1t")
        for c in range(4):
            nc.tensor.matmul(a1t_ps[:, 128 * c:128 * (c + 1)],
                             z1s[:, 128 * c:128 * (c + 1)], msbd[:],
                             start=True, stop=True)
        h1t = sb.tile([128, 512], BF16, tag="h1t")
        if has_b1:
            for c in range(4):
                nc.scalar.activation(
                    out=h1t[:, 128 * c:128 * (c + 1)],
                    in_=a1t_ps[:, 128 * c:128 * (c + 1)],
                    func=AF.Relu, bias=b1c[:, c:c + 1])
        else:
            nc.vector.tensor_scalar_max(h1t[:], a1t_ps[:], 0.0)

        # ---- z2 = h1 W2^T: [128 (g,n), 96 (s)] ----
        z2_ps = mps[:, 192:288]
        for c in range(4):
            nc.tensor.matmul(z2_ps, h1t[:, 128 * c:128 * (c + 1)],
                             w2t[:, 96 * c:96 * (c + 1)],
                             start=(c == 0), stop=(c == 3))
        z2s = sb.tile([128, 96], BF16, tag="z2s")
        nc.scalar.activation(out=z2s[:], in_=z2_ps, func=AF.Copy,
                             scale=dinv[:, :])

        # ---- agg2: [128 (g,n), 96 (l)] ----
        a2_ps = mps[:, 288:384]
        nc.tensor.matmul(a2_ps, msbd[:], z2s[:], start=True, stop=True)

        # ---- h2 duplicated [h2|h2] (+b2); a2 is already fully aggregated ----
        hp = sb.tile([128, 192], BF16, tag="hp")
        nc.vector.tensor_copy(out=hp[:, 0:96], in_=a2_ps)
        nc.scalar.activation(out=hp[:, 96:192], in_=a2_ps, func=AF.Copy)
        if has_b2:
            hpb = sb.tile([128, 192], BF16, tag="hpb")
            nc.vector.tensor_tensor(out=hpb[:], in0=hp[:], in1=b2d[:],
                                    op=mybir.AluOpType.add)
            hp = hpb

        # ---- conv: per graph 3 shifted-tap matmuls -> [96 (l), 512 (o)] ----
        for gl in range(2):
            po = 64 * gl
            y_ps = psy.tile([96, 512], FP32, tag="y")
            for k in range(3):
                tap = (95, 0, 1)[k]
                nc.tensor.matmul(
                    y_ps[:],
                    hp[po:po + 64, tap:tap + 96],
                    cwd[po:po + 64, 512 * k:512 * (k + 1)],
                    start=(k == 0), stop=(k == 2))
            ysb = sb.tile([96, 512], FP32, tag="ysb")
            if gl == 0:
                nc.vector.tensor_copy(out=ysb[:], in_=y_ps[:])
            else:
                nc.scalar.activation(out=ysb[:], in_=y_ps[:], func=AF.Copy)
            nc.sync.dma_start(out=y_d[2 * pr + gl], in_=ysb[:])

    ctx.close()


# ---------------- host side ----------------

def _prep_consts(W1, b1, W2, b2, conv_w):
    bf = ml_dtypes.bfloat16
    w1t = np.ascontiguousarray(W1.T).astype(bf)                    # [96, 512]
    w2t = np.ascontiguousarray(
        W2.T.reshape(4, 128, 96).transpose(1, 0, 2).reshape(128, 384)
    ).astype(bf)
    base = np.ascontiguousarray(conv_w.transpose(1, 2, 0)).reshape(64, 1536)
    cwd = np.concatenate([base, base], axis=0).astype(bf)          # [128, 1536]
    iota = np.broadcast_to((np.arange(1024) % 64).astype(bf), (128, 1024))
    iota = np.ascontiguousarray(iota)
    i64d = np.concatenate([np.eye(64), np.eye(64)], axis=0).astype(bf)
    id128 = np.eye(128).astype(bf)
    consts = dict(w1t=w1t, w2t=w2t, cwd=cwd, iota=iota, i64d=i64d,
                  id128=id128)
    has_b1 = bool(np.any(b1))
    has_b2 = bool(np.any(b2))
    if has_b1:
        consts["b1c"] = np.ascontiguousarray(
            b1.reshape(4, 128).T).astype(np.float32)
    if has_b2:
        b2d = np.ascontiguousarray(
            np.broadcast_to(np.tile(b2, 2).astype(bf), (128, 192)))
        consts["b2d"] = b2d
    return consts, has_b1, has_b2


_NC_CACHE = {}


def _get_nc(g_per_core, has_b1, has_b2):
    key = (g_per_core, has_b1, has_b2)
    if key in _NC_CACHE:
        return _NC_CACHE[key]
    nc = bacc.Bacc("TRN2", target_bir_lowering=False, debug=False)
    ins = {
        "x": nc.dram_tensor("x", [g_per_core, 96, 64], FP32,
                            kind="ExternalInput").ap(),
        "ei": nc.dram_tensor("ei", [2 * g_per_core, 512], I32,
                             kind="ExternalInput").ap(),
        "w1t": nc.dram_tensor("w1t", [96, 512], BF16,
                              kind="ExternalInput").ap(),
        "w2t": nc.dram_tensor("w2t", [128, 384], BF16,
                              kind="ExternalInput").ap(),
        "cwd": nc.dram_tensor("cwd", [128, 1536], BF16,
                              kind="ExternalInput").ap(),
        "iota": nc.dram_tensor("iota", [128, 1024], BF16,
                               kind="ExternalInput").ap(),
        "i64d": nc.dram_tensor("i64d", [128, 64], BF16,
                               kind="ExternalInput").ap(),
        "id128": nc.dram_tensor("id128", [128, 128], BF16,
                                kind="ExternalInput").ap(),
    }
    if has_b1:
        ins["b1c"] = nc.dram_tensor("b1c", [128, 4], FP32,
                                    kind="ExternalInput").ap()
    if has_b2:
        ins["b2d"] = nc.dram_tensor("b2d", [128, 192], BF16,
                                    kind="ExternalInput").ap()
    outs = {
        "y": nc.dram_tensor("y", [g_per_core, 96, 512], FP32,
                            kind="ExternalOutput").ap(),
    }
    with tile.TileContext(nc) as tc:
        build_gcn_kernel(tc, outs, ins, g_per_core, has_b1, has_b2)
    nc.compile()
    _NC_CACHE[key] = nc
    return nc


def kernel(x, edge_index, W1, b1, W2, b2, conv_w, _trace=False):
    x = np.asarray(x)
    edge_index = np.asarray(edge_index)
    consts, has_b1, has_b2 = _prep_consts(
        np.asarray(W1), np.asarray(b1), np.asarray(W2), np.asarray(b2),
        np.asarray(conv_w))
    nc = _get_nc(G, has_b1, has_b2)

    bfcast = {k: v for k, v in consts.items()}
    in_maps = []
    for c in range(N_CORES):
        sl = slice(c * G, (c + 1) * G)
        m = dict(bfcast)
        m["x"] = np.ascontiguousarray(x[sl]).astype(np.float32)
        m["ei"] = np.ascontiguousarray(
            edge_index[sl].reshape(2 * G, 512)).astype(np.int32)
        in_maps.append(m)

    res = run_bass_kernel_spmd(nc, in_maps, core_ids=list(range(N_CORES)),
                               trace=_trace)
    y = np.concatenate([res.results[c]["y"] for c in range(N_CORES)], axis=0)
    if _trace:
        kernel.last_results = res
    return y

